# revision 34
# baseline (speedup 1.0000x reference)
"""GP prediction kernel for Trainium2 (8 NeuronCores, data-parallel over batch).

Computes z_pred[b, p, d] = sum_c k_mult[p, c] * z_enc[b, c, d] where k_mult
is the [64, 448] GP weight matrix k_pred.T @ inv(cov + sigma*I). k_mult
depends only on compile-time constants, so it is precomputed on host; the
device work is a batched [64,448] @ [448,1024] matmul, sharded 8 batches
per core.

Mixed precision against the 2e-2 correctness gate: the PE supports
mixed-dtype matmuls (fp16 stationary x fp8 moving, HW-verified exact), so
ALL weights stay fp16 (zero weight quantization error) and only z is
quantized: the 32 highest-energy context steps travel fp16, the other 416
as unscaled fp8-e4m3. End-to-end error ~1.31e-2, all of it from fp8(z).

PE layout: weights are only 64 wide, so each matmul uses half the
128-wide array; the two batches of a pair run CONCURRENTLY via PE column
tiling (h=0 in array cols 0-63 -> PSUM partitions 0-63, h=1 in cols
64-127) with separate XBUS streams, halving PE streaming time. The two
32-row K-tiles (fp16 + fp8 leftover) pack FOUR batches (two pairs) into
one 128-partition tile on disjoint 32-row groups, so those matmuls also
run on disjoint sub-arrays. PSUM accumulates the 5 K-tiles (32 fp16 +
3x128 fp8 + 32 fp8) into a [128, 512] bank per (pair, 512-col half); one
DVE copy casts each to fp16.

DMA: whole-tile transfers only (768KB fp8 tiles / 256KB quad tiles) --
fine-splitting the stream was measured strictly worse (per-DMA overhead
and HAM-cooling PE stalls). Weights go on the gpsimd (SWDGE) queue so
sync/scalar start streaming z immediately after the framework preamble;
outputs for pairs 0-2 also ride gpsimd, the last pair's output splits
across sync+scalar. Pairs 2,3 consume their fp8 K-tiles FIRST (psum
group starts on q0) so the late-arriving quad-1 fp16 tile only gates two
cheap 32-row matmul slots at the very end. Warm-up matmuls (memset on
DVE) keep the PE busy from the end of the preamble so the HAM clock-gate
lifts to 2.4 GHz before the real matmuls start.
"""
import numpy as np
from contextlib import ExitStack

import concourse.bacc as bacc
import concourse.tile as tile
from concourse import mybir
from concourse.bass_utils import run_bass_kernel_spmd

# Problem constants (hardcoded per harness contract).
B, T, D = 64, 512, 1024
P = 64                 # N_PREDICTORS
C = T - P              # 448 context timesteps
L, SIGMA, TIMESCALE = 0.01, 0.01, 0.3
N_CORES = 8
BPC = B // N_CORES     # batches per core
NPAIR = BPC // 2       # batch pairs per core

N16 = 32               # high-energy columns in fp16 (one 32-row K-tile)
N8 = C - N16           # low-energy columns in fp8: 3x128 + one 32-row tile
NWARM = 7              # garbage matmuls to pull the HAM clock-gate early

F8 = mybir.dt.np(mybir.dt.float8e4)   # ml_dtypes.float8_e4m3


def _k_mult() -> np.ndarray:
    """[P, C] GP weight matrix, solved in float64 on host."""
    t = np.linspace(0.0, 1.0, T)
    t_in = t[:C] * TIMESCALE
    t_pred = t[C:] * TIMESCALE

    def rbf_np(x, y):
        d = x[:, None] - y[None, :]
        return np.exp(-0.5 * d * d / L)

    cov = rbf_np(t_in, t_in) + np.eye(C) * SIGMA
    return np.linalg.solve(cov, rbf_np(t_in, t_pred)).T   # [P, C] float64


def _prep_constants():
    km = _k_mult()
    energy = (km * km).sum(axis=0)
    order = np.argsort(energy)
    cols8 = np.sort(order[:N8])        # fp8 columns, natural order
    cols16 = np.sort(order[N8:])       # fp16 columns (32)
    cols8big = cols8[: 3 * 128]        # the three 128-row fp8 K-tiles
    cols8x = cols8[3 * 128 :]          # leftover 32-row fp8 K-tile

    def quad(w):                       # [32, P] -> [128, P], 4 row-copies
        return np.ascontiguousarray(np.tile(w, (4, 1)).astype(np.float16))

    km16q = quad(km[:, cols16].T)      # [128, P] fp16
    km8xq = quad(km[:, cols8x].T)      # [128, P] fp16
    km8 = np.zeros((128, 3 * P), np.float16)
    for q in range(3):
        km8[:, q * P : (q + 1) * P] = km[:, cols8big[q * 128 : (q + 1) * 128]].T
    return cols16, cols8big, cols8x, km16q, km8xq, km8


COLS16, COLS8BIG, COLS8X, KM16Q, KM8XQ, KM8_DEV = _prep_constants()

_NC = None


def _build():
    nc = bacc.Bacc()
    # per quad (2 pairs): [128, D], partition b*32+k = batch 4i+b, K-row k
    zj16 = nc.dram_tensor("zj16", [(NPAIR // 2) * 128, D], mybir.dt.float16,
                          kind="ExternalInput")
    z8x = nc.dram_tensor("z8x", [(NPAIR // 2) * 128, D], mybir.dt.float8e4,
                         kind="ExternalInput")
    # per pair: [128, 6*D] fp8, col q*2D + h*D + d = subtile q, batch h
    # (q-major so a pair splits into a 512KB q0+q1 piece and a 256KB q2
    # piece whose completion sems stagger ~2us apart)
    z8 = nc.dram_tensor("z8", [NPAIR * 128, 6 * D], mybir.dt.float8e4,
                        kind="ExternalInput")
    km16 = nc.dram_tensor("km16", [128, P], mybir.dt.float16,
                          kind="ExternalInput")
    km8x = nc.dram_tensor("km8x", [128, P], mybir.dt.float16,
                          kind="ExternalInput")
    km8 = nc.dram_tensor("km8", [128, 3 * P], mybir.dt.float16,
                         kind="ExternalInput")
    # per pair: [128, D] fp16, partition h*64+p = batch 2p+h, predictor p
    out = nc.dram_tensor("out", [NPAIR * 128, D], mybir.dt.float16,
                         kind="ExternalOutput")

    with tile.TileContext(nc) as tc, ExitStack() as ctx:
        kpool = ctx.enter_context(tc.tile_pool(name="km", bufs=1))
        wpool = ctx.enter_context(tc.tile_pool(name="warm", bufs=1))
        zqpool = ctx.enter_context(tc.tile_pool(name="zq", bufs=NPAIR))
        z8pool = ctx.enter_context(tc.tile_pool(name="z8", bufs=NPAIR))
        opool = ctx.enter_context(tc.tile_pool(name="o", bufs=NPAIR))
        ppool = ctx.enter_context(tc.tile_pool(name="ps", bufs=7, space="PSUM"))
        wppool = ctx.enter_context(tc.tile_pool(name="wps", bufs=1, space="PSUM"))

        # Warm-up: garbage matmuls with no data dependencies so the HAM
        # clock-gate lifts the 1.2 GHz cap before the real matmuls start.
        warm = wpool.tile([128, P + 512], mybir.dt.float16)
        nc.vector.memset(warm[:, :], 1.0)
        wps = wppool.tile([P, 512], mybir.dt.float32)
        for _ in range(NWARM):
            nc.tensor.matmul(wps[:, :], warm[:, :P], warm[:, P : P + 512],
                             start=True, stop=True)

        km16_sb = kpool.tile([128, P], mybir.dt.float16)
        km8x_sb = kpool.tile([128, P], mybir.dt.float16)
        km8_sb = kpool.tile([128, 3 * P], mybir.dt.float16)
        # weights ride gpsimd: keeps their completion receipts off the two
        # HWDGE z-queues (completion sems serialize per queue and lag the
        # data by 1-2.5us under HBM load)
        nc.gpsimd.dma_start(km16_sb[:, :], km16[:, :])
        nc.gpsimd.dma_start(km8x_sb[:, :], km8x[:, :])
        nc.gpsimd.dma_start(km8_sb[:, :], km8[:, :])

        z16t = [zqpool.tile([128, D], mybir.dt.float16,
                            name=f"z16_{i}", tag="z16")
                for i in range(NPAIR // 2)]
        z8xt = [zqpool.tile([128, D], mybir.dt.float8e4,
                            name=f"z8x_{i}", tag="z8x")
                for i in range(NPAIR // 2)]
        z8t = [z8pool.tile([128, 6 * D], mybir.dt.float8e4,
                           name=f"z8_{pr}", tag="z8")
               for pr in range(NPAIR)]

        def _r(pr):
            return slice(pr * 128, (pr + 1) * 128)

        # Input schedule: whole tiles, pair-ordered, both queues balanced at
        # ~1.92MB. Quad tiles (fp16 + fp8x) all land by ~13.5us so the cheap
        # 32-row matmuls can bridge PE gaps; the last items are the big z8
        # tiles of pairs 2/3.
        # Each z8 tile rides ONE queue as a 512KB (q0,q1) piece + 256KB q2
        # piece: staggered completion sems let each pair's q0/q1 matmuls
        # start ~2us before the whole tile's receipt would fire.
        nc.scalar.dma_start(z8xt[0][:, :], z8x[_r(0), :])
        nc.scalar.dma_start(z16t[0][:, :], zj16[_r(0), :])
        nc.sync.dma_start(z8t[0][:, : 4 * D], z8[_r(0), : 4 * D])
        nc.sync.dma_start(z8t[0][:, 4 * D :], z8[_r(0), 4 * D :])
        nc.sync.dma_start(z16t[1][:, :], zj16[_r(1), :])
        nc.sync.dma_start(z8xt[1][:, :], z8x[_r(1), :])
        nc.scalar.dma_start(z8t[1][:, : 4 * D], z8[_r(1), : 4 * D])
        nc.scalar.dma_start(z8t[1][:, 4 * D :], z8[_r(1), 4 * D :])
        nc.sync.dma_start(z8t[2][:, : 4 * D], z8[_r(2), : 4 * D])
        nc.scalar.dma_start(z8t[3][:, : 4 * D], z8[_r(3), : 4 * D])
        nc.sync.dma_start(z8t[2][:, 4 * D :], z8[_r(2), 4 * D :])
        nc.scalar.dma_start(z8t[3][:, 4 * D :], z8[_r(3), 4 * D :])

        def _mm_small(pr, n, which, ps, start, stop=False):
            # 32-row K-tiles: 4 batches of the quad on disjoint 32-row
            # groups; h=0/1 of this pair also on disjoint column groups.
            i, t = pr // 2, pr % 2
            src = z16t[i] if which == 0 else z8xt[i]
            w = km16_sb if which == 0 else km8x_sb
            for h in range(2):
                b = t * 2 + h
                bs = slice(b * 32, (b + 1) * 32)
                nc.tensor.matmul(ps[h * P : (h + 1) * P, :], w[bs, :],
                                 src[bs, n * 512 : (n + 1) * 512],
                                 start=start, stop=stop,
                                 skip_group_check=True,
                                 tile_position=(b * 32, h * P))

        def _mm_big(pr, n, q, ps, start, stop):
            # 128-row fp8 K-tile: h=0/1 in column groups 0-63/64-127 with
            # separate XBUS streams -> concurrent
            for h in range(2):
                rhs = z8t[pr][:, q * 2 * D + h * D + n * 512 :
                              q * 2 * D + h * D + (n + 1) * 512]
                nc.tensor.matmul(ps[h * P : (h + 1) * P, :],
                                 km8_sb[:, q * P : (q + 1) * P], rhs,
                                 start=start, stop=stop,
                                 skip_group_check=True)

        # PE emission in two waves of two pairs. Within a wave, ALL small
        # 32-row matmuls go first (their quad tiles land early, keeping the
        # PE continuously busy so the HAM clock-gate stays at 2.4 GHz), then
        # the big fp8 matmuls chase their tiles as they land. Groups open on
        # the first small matmul and close on q2.
        def _warm_fill(k):
            # sprinkled warm-filler: the PE must NEVER see an idle activity
            # window or the HAM clock-gate halves its clock; these absorb
            # DMA-sem waits at 215ns each and are nearly free when data is
            # ready (PE has 2x headroom over the DMA stream)
            for _ in range(k):
                nc.tensor.matmul(wps[:, :], warm[:, :P],
                                 warm[:, P : P + 512],
                                 start=True, stop=True)

        out_sbs = [opool.tile([128, D], mybir.dt.float16,
                              name=f"o_{pr}", tag="o")
                   for pr in range(NPAIR)]
        pss = {}
        for wave in range(2):
            prs = (0, 1) if wave == 0 else (2, 3)
            for pr in prs:
                for n in range(2):
                    ps = ppool.tile([128, 512], mybir.dt.float32,
                                    name=f"ps{pr}_{n}", tag="ps")
                    pss[pr, n] = ps
                    _mm_small(pr, n, 0, ps, start=True)
                    _mm_small(pr, n, 1, ps, start=False)
            if wave == 0:
                # bridge: the smalls end ~2us before the first big piece's
                # completion sem; keep the PE busy or the HAM clock-gate
                # drops it to 1.2 GHz for the whole mid-phase
                _warm_fill(9)
            for pr in prs:
                # q0/q1 matmuls of both n-halves first (their 512KB piece's
                # sem fires ~2us before the q2 piece's), q2 closes each group
                for n in range(2):
                    for q in range(2):
                        _mm_big(pr, n, q, pss[pr, n], start=False, stop=False)
                for n in range(2):
                    ps = pss[pr, n]
                    _mm_big(pr, n, 2, ps, start=False, stop=True)
                    if pr == NPAIR - 1 and n == 1:
                        # last cast on scalar ACTIVATE so it overlaps the
                        # DVE's pr3-n0 cast; emitted late so the one-time
                        # ACT table load schedules into scalar's mid-stream
                        # idle, not ahead of its z DMAs
                        nc.scalar.activation(
                            out_sbs[pr][:, 512:], ps[:, :],
                            mybir.ActivationFunctionType.Copy)
                    else:
                        nc.vector.tensor_copy(
                            out_sbs[pr][:, n * 512 : (n + 1) * 512], ps[:, :])
                if pr < NPAIR - 1:
                    nc.gpsimd.dma_start(out[_r(pr), :], out_sbs[pr][:, :])
                else:
                    # last pair: split across the two HWDGE queues
                    nc.sync.dma_start(out[_r(pr), :512], out_sbs[pr][:, :512])
                    nc.scalar.dma_start(out[_r(pr), 512:], out_sbs[pr][:, 512:])

    nc.finalize()
    return nc


def kernel(z_enc: np.ndarray, _trace: bool = False):
    global _NC
    z_enc = np.asarray(z_enc, dtype=np.float32)
    if _NC is None:
        _NC = _build()

    zc = z_enc[:, :C, :]
    # quad tiles: [B, 32, D] -> [B/4 * 128, D] (batch-of-quad outer, k inner)
    zj16 = np.ascontiguousarray(
        zc[:, COLS16, :].astype(np.float16).reshape(B // 4 * 128, D))
    z8x = np.ascontiguousarray(
        zc[:, COLS8X, :].astype(F8).reshape(B // 4 * 128, D))

    # big fp8 tile, q-major: [B,384,D] -> [B/2, 2(h), 3(q), 128(k), D]
    #            -> [B/2, 128(k), 3(q), 2(h), D] -> [B/2*128, 6D]
    z8 = np.ascontiguousarray(
        zc[:, COLS8BIG, :].reshape(B // 2, 2, 3, 128, D)
        .transpose(0, 3, 2, 1, 4)
    ).astype(F8).reshape(B // 2 * 128, 6 * D)

    rows = NPAIR * 128
    qrows = (NPAIR // 2) * 128
    in_maps = [
        {
            "zj16": zj16[i * qrows : (i + 1) * qrows],
            "z8x": z8x[i * qrows : (i + 1) * qrows],
            "z8": z8[i * rows : (i + 1) * rows],
            "km16": KM16Q,
            "km8x": KM8XQ,
            "km8": KM8_DEV,
        }
        for i in range(N_CORES)
    ]

    res = run_bass_kernel_spmd(_NC, in_maps, core_ids=list(range(N_CORES)),
                               trace=_trace)
    # out rows: pair-major, partition h*64+p -> [B/2, 2, 64, D] == [B, 64, D]
    out = np.concatenate(
        [r["out"].reshape(BPC // 2, 128, D) for r in res.results], axis=0
    ).reshape(B, P, D).astype(np.float32)
    if _trace:
        return out, res
    return out


# revision 36
# speedup vs baseline: 1.1022x; 1.1022x over previous
"""GP prediction kernel for Trainium2 (8 NeuronCores, data-parallel over batch).

Computes z_pred[b, p, d] = sum_c k_mult[p, c] * z_enc[b, c, d] where k_mult
is the [64, 448] GP weight matrix k_pred.T @ inv(cov + sigma*I). k_mult
depends only on compile-time constants, so it is precomputed on host; the
device work is a batched [64,448] @ [448,1024] matmul, sharded 8 batches
per core.

Mixed precision against the 2e-2 correctness gate: the PE supports
mixed-dtype matmuls (fp16 stationary x fp8 moving, HW-verified exact), so
ALL weights stay fp16 (zero weight quantization error) and only z is
quantized: the 32 highest-energy context steps travel fp16, the other 416
as unscaled fp8-e4m3. End-to-end error ~1.31e-2, all of it from fp8(z).

PE layout: weights are only 64 wide, so each matmul uses half the
128-wide array; the two batches of a pair run CONCURRENTLY via PE column
tiling (h=0 in array cols 0-63 -> PSUM partitions 0-63, h=1 in cols
64-127) with separate XBUS streams, halving PE streaming time. The two
32-row K-tiles (fp16 + fp8 leftover) pack FOUR batches (two pairs) into
one 128-partition tile on disjoint 32-row groups, so those matmuls also
run on disjoint sub-arrays. PSUM accumulates the 5 K-tiles (32 fp16 +
3x128 fp8 + 32 fp8) into a [128, 512] bank per (pair, 512-col half); one
DVE copy casts each to fp16.

DMA: whole-tile transfers only (768KB fp8 tiles / 256KB quad tiles) --
fine-splitting the stream was measured strictly worse (per-DMA overhead
and HAM-cooling PE stalls). Weights go on the gpsimd (SWDGE) queue so
sync/scalar start streaming z immediately after the framework preamble;
outputs for pairs 0-2 also ride gpsimd, the last pair's output splits
across sync+scalar. Pairs 2,3 consume their fp8 K-tiles FIRST (psum
group starts on q0) so the late-arriving quad-1 fp16 tile only gates two
cheap 32-row matmul slots at the very end. Warm-up matmuls (memset on
DVE) keep the PE busy from the end of the preamble so the HAM clock-gate
lifts to 2.4 GHz before the real matmuls start.
"""
import numpy as np
from contextlib import ExitStack

import concourse.bacc as bacc
import concourse.tile as tile
from concourse import mybir
from concourse.bass_utils import run_bass_kernel_spmd

# Problem constants (hardcoded per harness contract).
B, T, D = 64, 512, 1024
P = 64                 # N_PREDICTORS
C = T - P              # 448 context timesteps
L, SIGMA, TIMESCALE = 0.01, 0.01, 0.3
N_CORES = 8
BPC = B // N_CORES     # batches per core
NPAIR = BPC // 2       # batch pairs per core

N16 = 32               # high-energy columns in fp16 (one 32-row K-tile)
N8 = C - N16           # low-energy columns in fp8: 3x128 + one 32-row tile
NWARM = 7              # garbage matmuls to pull the HAM clock-gate early

F8 = mybir.dt.np(mybir.dt.float8e4)   # ml_dtypes.float8_e4m3


def _k_mult() -> np.ndarray:
    """[P, C] GP weight matrix, solved in float64 on host."""
    t = np.linspace(0.0, 1.0, T)
    t_in = t[:C] * TIMESCALE
    t_pred = t[C:] * TIMESCALE

    def rbf_np(x, y):
        d = x[:, None] - y[None, :]
        return np.exp(-0.5 * d * d / L)

    cov = rbf_np(t_in, t_in) + np.eye(C) * SIGMA
    return np.linalg.solve(cov, rbf_np(t_in, t_pred)).T   # [P, C] float64


def _prep_constants():
    km = _k_mult()
    energy = (km * km).sum(axis=0)
    order = np.argsort(energy)
    cols8 = np.sort(order[:N8])        # fp8 columns, natural order
    cols16 = np.sort(order[N8:])       # fp16 columns (32)
    cols8big = cols8[: 3 * 128]        # the three 128-row fp8 K-tiles
    cols8x = cols8[3 * 128 :]          # leftover 32-row fp8 K-tile

    def quad(w):                       # [32, P] -> [128, P], 4 row-copies
        return np.ascontiguousarray(np.tile(w, (4, 1)).astype(np.float16))

    km16q = quad(km[:, cols16].T)      # [128, P] fp16
    km8xq = quad(km[:, cols8x].T)      # [128, P] fp16
    km8 = np.zeros((128, 3 * P), np.float16)
    for q in range(3):
        km8[:, q * P : (q + 1) * P] = km[:, cols8big[q * 128 : (q + 1) * 128]].T
    return cols16, cols8big, cols8x, km16q, km8xq, km8


COLS16, COLS8BIG, COLS8X, KM16Q, KM8XQ, KM8_DEV = _prep_constants()

_NC = None


def _build():
    nc = bacc.Bacc()
    # per quad (2 pairs): [128, D], partition b*32+k = batch 4i+b, K-row k
    zj16 = nc.dram_tensor("zj16", [(NPAIR // 2) * 128, D], mybir.dt.float16,
                          kind="ExternalInput")
    z8x = nc.dram_tensor("z8x", [(NPAIR // 2) * 128, D], mybir.dt.float8e4,
                         kind="ExternalInput")
    # per pair: [128, 6*D] fp8, col q*2D + h*D + d = subtile q, batch h
    # (q-major so a pair splits into a 512KB q0+q1 piece and a 256KB q2
    # piece whose completion sems stagger ~2us apart)
    z8 = nc.dram_tensor("z8", [NPAIR * 128, 6 * D], mybir.dt.float8e4,
                        kind="ExternalInput")
    km16 = nc.dram_tensor("km16", [128, P], mybir.dt.float16,
                          kind="ExternalInput")
    km8x = nc.dram_tensor("km8x", [128, P], mybir.dt.float16,
                          kind="ExternalInput")
    km8 = nc.dram_tensor("km8", [128, 3 * P], mybir.dt.float16,
                         kind="ExternalInput")
    # per pair: [128, D] fp16, partition h*64+p = batch 2p+h, predictor p
    out = nc.dram_tensor("out", [NPAIR * 128, D], mybir.dt.float16,
                         kind="ExternalOutput")

    with tile.TileContext(nc) as tc, ExitStack() as ctx:
        kpool = ctx.enter_context(tc.tile_pool(name="km", bufs=1))
        wpool = ctx.enter_context(tc.tile_pool(name="warm", bufs=1))
        zqpool = ctx.enter_context(tc.tile_pool(name="zq", bufs=NPAIR))
        z8pool = ctx.enter_context(tc.tile_pool(name="z8", bufs=NPAIR))
        opool = ctx.enter_context(tc.tile_pool(name="o", bufs=NPAIR))
        ppool = ctx.enter_context(tc.tile_pool(name="ps", bufs=7, space="PSUM"))
        wppool = ctx.enter_context(tc.tile_pool(name="wps", bufs=1, space="PSUM"))

        # Warm-up: garbage matmuls with no data dependencies so the HAM
        # clock-gate lifts the 1.2 GHz cap before the real matmuls start.
        warm = wpool.tile([128, P + 512], mybir.dt.float16)
        nc.vector.memset(warm[:, :], 1.0)
        wps = wppool.tile([P, 512], mybir.dt.float32)
        for _ in range(NWARM):
            nc.tensor.matmul(wps[:, :], warm[:, :P], warm[:, P : P + 512],
                             start=True, stop=True)

        km16_sb = kpool.tile([128, P], mybir.dt.float16)
        km8x_sb = kpool.tile([128, P], mybir.dt.float16)
        km8_sb = kpool.tile([128, 3 * P], mybir.dt.float16)
        # weights ride gpsimd: keeps their completion receipts off the two
        # HWDGE z-queues (completion sems serialize per queue and lag the
        # data by 1-2.5us under HBM load)
        nc.gpsimd.dma_start(km16_sb[:, :], km16[:, :])
        nc.gpsimd.dma_start(km8x_sb[:, :], km8x[:, :])
        nc.gpsimd.dma_start(km8_sb[:, :], km8[:, :])

        z16t = [zqpool.tile([128, D], mybir.dt.float16,
                            name=f"z16_{i}", tag="z16")
                for i in range(NPAIR // 2)]
        z8xt = [zqpool.tile([128, D], mybir.dt.float8e4,
                            name=f"z8x_{i}", tag="z8x")
                for i in range(NPAIR // 2)]
        z8t = [z8pool.tile([128, 6 * D], mybir.dt.float8e4,
                           name=f"z8_{pr}", tag="z8")
               for pr in range(NPAIR)]

        def _r(pr):
            return slice(pr * 128, (pr + 1) * 128)

        # Input schedule: whole tiles, pair-ordered, both queues balanced at
        # ~1.92MB. Quad tiles (fp16 + fp8x) all land by ~13.5us so the cheap
        # 32-row matmuls can bridge PE gaps; the last items are the big z8
        # tiles of pairs 2/3.
        # Each z8 tile rides ONE queue as a 512KB (q0,q1) piece + 256KB q2
        # piece: staggered completion sems let each pair's q0/q1 matmuls
        # start ~2us before the whole tile's receipt would fire.
        nc.scalar.dma_start(z8xt[0][:, :], z8x[_r(0), :])
        nc.scalar.dma_start(z16t[0][:, :], zj16[_r(0), :])
        nc.sync.dma_start(z8t[0][:, : 4 * D], z8[_r(0), : 4 * D])
        nc.sync.dma_start(z8t[0][:, 4 * D :], z8[_r(0), 4 * D :])
        nc.sync.dma_start(z16t[1][:, :], zj16[_r(1), :])
        nc.sync.dma_start(z8xt[1][:, :], z8x[_r(1), :])
        nc.scalar.dma_start(z8t[1][:, : 4 * D], z8[_r(1), : 4 * D])
        nc.scalar.dma_start(z8t[1][:, 4 * D :], z8[_r(1), 4 * D :])
        nc.sync.dma_start(z8t[2][:, : 4 * D], z8[_r(2), : 4 * D])
        nc.scalar.dma_start(z8t[3][:, : 4 * D], z8[_r(3), : 4 * D])
        nc.sync.dma_start(z8t[2][:, 4 * D :], z8[_r(2), 4 * D :])
        nc.scalar.dma_start(z8t[3][:, 4 * D :], z8[_r(3), 4 * D :])

        def _mm_small(pr, n, which, ps, start, stop=False):
            # 32-row K-tiles: 4 batches of the quad on disjoint 32-row
            # groups; h=0/1 of this pair also on disjoint column groups.
            i, t = pr // 2, pr % 2
            src = z16t[i] if which == 0 else z8xt[i]
            w = km16_sb if which == 0 else km8x_sb
            for h in range(2):
                b = t * 2 + h
                bs = slice(b * 32, (b + 1) * 32)
                nc.tensor.matmul(ps[h * P : (h + 1) * P, :], w[bs, :],
                                 src[bs, n * 512 : (n + 1) * 512],
                                 start=start, stop=stop,
                                 skip_group_check=True,
                                 tile_position=(b * 32, h * P))

        def _mm_big(pr, n, q, ps, start, stop):
            # 128-row fp8 K-tile: h=0/1 in column groups 0-63/64-127 with
            # separate XBUS streams -> concurrent
            for h in range(2):
                rhs = z8t[pr][:, q * 2 * D + h * D + n * 512 :
                              q * 2 * D + h * D + (n + 1) * 512]
                nc.tensor.matmul(ps[h * P : (h + 1) * P, :],
                                 km8_sb[:, q * P : (q + 1) * P], rhs,
                                 start=start, stop=stop,
                                 skip_group_check=True)

        # PE emission in two waves of two pairs. Within a wave, ALL small
        # 32-row matmuls go first (their quad tiles land early, keeping the
        # PE continuously busy so the HAM clock-gate stays at 2.4 GHz), then
        # the big fp8 matmuls chase their tiles as they land. Groups open on
        # the first small matmul and close on q2.
        out_sbs = [opool.tile([128, D], mybir.dt.float16,
                              name=f"o_{pr}", tag="o")
                   for pr in range(NPAIR)]
        pss = {}
        for wave in range(2):
            prs = (0, 1) if wave == 0 else (2, 3)
            for pr in prs:
                for n in range(2):
                    ps = ppool.tile([128, 512], mybir.dt.float32,
                                    name=f"ps{pr}_{n}", tag="ps")
                    pss[pr, n] = ps
                    _mm_small(pr, n, 0, ps, start=True)
                    _mm_small(pr, n, 1, ps, start=False)
            for pr in prs:
                # q0/q1 matmuls of both n-halves first (their 512KB piece's
                # sem fires ~2us before the q2 piece's), q2 closes each group
                for n in range(2):
                    for q in range(2):
                        _mm_big(pr, n, q, pss[pr, n], start=False, stop=False)
                for n in range(2):
                    ps = pss[pr, n]
                    _mm_big(pr, n, 2, ps, start=False, stop=True)
                    if pr == NPAIR - 1 and n == 1:
                        # last cast on scalar ACTIVATE so it overlaps the
                        # DVE's pr3-n0 cast; emitted late so the one-time
                        # ACT table load schedules into scalar's mid-stream
                        # idle, not ahead of its z DMAs
                        nc.scalar.activation(
                            out_sbs[pr][:, 512:], ps[:, :],
                            mybir.ActivationFunctionType.Copy)
                    else:
                        nc.vector.tensor_copy(
                            out_sbs[pr][:, n * 512 : (n + 1) * 512], ps[:, :])
                if pr < NPAIR - 1:
                    nc.gpsimd.dma_start(out[_r(pr), :], out_sbs[pr][:, :])
                else:
                    # last pair: split across the two HWDGE queues
                    nc.sync.dma_start(out[_r(pr), :512], out_sbs[pr][:, :512])
                    nc.scalar.dma_start(out[_r(pr), 512:], out_sbs[pr][:, 512:])

    nc.finalize()
    return nc


def kernel(z_enc: np.ndarray, _trace: bool = False):
    global _NC
    z_enc = np.asarray(z_enc, dtype=np.float32)
    if _NC is None:
        _NC = _build()

    zc = z_enc[:, :C, :]
    # quad tiles: [B, 32, D] -> [B/4 * 128, D] (batch-of-quad outer, k inner)
    zj16 = np.ascontiguousarray(
        zc[:, COLS16, :].astype(np.float16).reshape(B // 4 * 128, D))
    z8x = np.ascontiguousarray(
        zc[:, COLS8X, :].astype(F8).reshape(B // 4 * 128, D))

    # big fp8 tile, q-major: [B,384,D] -> [B/2, 2(h), 3(q), 128(k), D]
    #            -> [B/2, 128(k), 3(q), 2(h), D] -> [B/2*128, 6D]
    z8 = np.ascontiguousarray(
        zc[:, COLS8BIG, :].reshape(B // 2, 2, 3, 128, D)
        .transpose(0, 3, 2, 1, 4)
    ).astype(F8).reshape(B // 2 * 128, 6 * D)

    rows = NPAIR * 128
    qrows = (NPAIR // 2) * 128
    in_maps = [
        {
            "zj16": zj16[i * qrows : (i + 1) * qrows],
            "z8x": z8x[i * qrows : (i + 1) * qrows],
            "z8": z8[i * rows : (i + 1) * rows],
            "km16": KM16Q,
            "km8x": KM8XQ,
            "km8": KM8_DEV,
        }
        for i in range(N_CORES)
    ]

    res = run_bass_kernel_spmd(_NC, in_maps, core_ids=list(range(N_CORES)),
                               trace=_trace)
    # out rows: pair-major, partition h*64+p -> [B/2, 2, 64, D] == [B, 64, D]
    out = np.concatenate(
        [r["out"].reshape(BPC // 2, 128, D) for r in res.results], axis=0
    ).reshape(B, P, D).astype(np.float32)
    if _trace:
        return out, res
    return out


# revision 37
# speedup vs baseline: 1.1504x; 1.0437x over previous
"""GP prediction kernel for Trainium2 (8 NeuronCores, data-parallel over batch).

Computes z_pred[b, p, d] = sum_c k_mult[p, c] * z_enc[b, c, d] where k_mult
is the [64, 448] GP weight matrix k_pred.T @ inv(cov + sigma*I). k_mult
depends only on compile-time constants, so it is precomputed on host; the
device work is a batched [64,448] @ [448,1024] matmul, sharded 8 batches
per core.

Mixed precision against the 2e-2 correctness gate: the PE supports
mixed-dtype matmuls (fp16 stationary x fp8 moving, HW-verified exact), so
ALL weights stay fp16 (zero weight quantization error) and only z is
quantized: the 32 highest-energy context steps travel fp16, the other 416
as unscaled fp8-e4m3. End-to-end error ~1.31e-2, all of it from fp8(z).

PE layout: weights are only 64 wide, so each matmul uses half the
128-wide array; the two batches of a pair run CONCURRENTLY via PE column
tiling (h=0 in array cols 0-63 -> PSUM partitions 0-63, h=1 in cols
64-127) with separate XBUS streams, halving PE streaming time. The two
32-row K-tiles (fp16 + fp8 leftover) pack FOUR batches (two pairs) into
one 128-partition tile on disjoint 32-row groups, so those matmuls also
run on disjoint sub-arrays. PSUM accumulates the 5 K-tiles (32 fp16 +
3x128 fp8 + 32 fp8) into a [128, 512] bank per (pair, 512-col half); one
DVE copy casts each to fp16.

DMA: big pair-ordered transfers on the two HWDGE queues (sync/scalar),
balanced at ~1.92MB each. DMA completion semaphores lag the last byte by
1-2.5us under HBM load (write-receipt round trip), so each 768KB fp8
tile is laid out q-major and sent as a 512KB (q0,q1) piece plus a 256KB
q2 piece on ONE queue: the first piece's sem fires ~2us earlier and the
pair's q0/q1 matmuls overlap the remaining stream. Fine-splitting beyond
this was measured strictly worse. Weights ride the gpsimd (SWDGE) queue
(keeps their receipts off the z-queues), as do outputs for pairs 0-2;
the last pair's output splits across sync+scalar, and its second cast
runs on scalar ACTIVATE to overlap the DVE cast. Warm-up matmuls
(memset on DVE) keep the PE busy from the end of the framework preamble
so the HAM clock-gate lifts to 2.4 GHz before the real matmuls start.
"""
import numpy as np
from contextlib import ExitStack

import concourse.bacc as bacc
import concourse.tile as tile
from concourse import mybir
from concourse.bass_utils import run_bass_kernel_spmd

# Problem constants (hardcoded per harness contract).
B, T, D = 64, 512, 1024
P = 64                 # N_PREDICTORS
C = T - P              # 448 context timesteps
L, SIGMA, TIMESCALE = 0.01, 0.01, 0.3
N_CORES = 8
BPC = B // N_CORES     # batches per core
NPAIR = BPC // 2       # batch pairs per core

N16 = 32               # high-energy columns in fp16 (one 32-row K-tile)
N8 = C - N16           # low-energy columns in fp8: 3x128 + one 32-row tile
NWARM = 7              # garbage matmuls to pull the HAM clock-gate early

F8 = mybir.dt.np(mybir.dt.float8e4)   # ml_dtypes.float8_e4m3


def _k_mult() -> np.ndarray:
    """[P, C] GP weight matrix, solved in float64 on host."""
    t = np.linspace(0.0, 1.0, T)
    t_in = t[:C] * TIMESCALE
    t_pred = t[C:] * TIMESCALE

    def rbf_np(x, y):
        d = x[:, None] - y[None, :]
        return np.exp(-0.5 * d * d / L)

    cov = rbf_np(t_in, t_in) + np.eye(C) * SIGMA
    return np.linalg.solve(cov, rbf_np(t_in, t_pred)).T   # [P, C] float64


def _prep_constants():
    km = _k_mult()
    energy = (km * km).sum(axis=0)
    order = np.argsort(energy)
    cols8 = np.sort(order[:N8])        # fp8 columns, natural order
    cols16 = np.sort(order[N8:])       # fp16 columns (32)
    cols8big = cols8[: 3 * 128]        # the three 128-row fp8 K-tiles
    cols8x = cols8[3 * 128 :]          # leftover 32-row fp8 K-tile

    def quad(w):                       # [32, P] -> [128, P], 4 row-copies
        return np.ascontiguousarray(np.tile(w, (4, 1)).astype(np.float16))

    km16q = quad(km[:, cols16].T)      # [128, P] fp16
    km8xq = quad(km[:, cols8x].T)      # [128, P] fp16
    km8 = np.zeros((128, 3 * P), np.float16)
    for q in range(3):
        km8[:, q * P : (q + 1) * P] = km[:, cols8big[q * 128 : (q + 1) * 128]].T
    return cols16, cols8big, cols8x, km16q, km8xq, km8


COLS16, COLS8BIG, COLS8X, KM16Q, KM8XQ, KM8_DEV = _prep_constants()

_NC = None


def _build():
    nc = bacc.Bacc()
    # per quad (2 pairs): [128, D], partition b*32+k = batch 4i+b, K-row k
    zj16 = nc.dram_tensor("zj16", [(NPAIR // 2) * 128, D], mybir.dt.float16,
                          kind="ExternalInput")
    z8x = nc.dram_tensor("z8x", [(NPAIR // 2) * 128, D], mybir.dt.float8e4,
                         kind="ExternalInput")
    # per pair: [128, 6*D] fp8, col q*2D + h*D + d = subtile q, batch h
    # (q-major so a pair splits into a 512KB q0+q1 piece and a 256KB q2
    # piece whose completion sems stagger ~2us apart)
    z8 = nc.dram_tensor("z8", [NPAIR * 128, 6 * D], mybir.dt.float8e4,
                        kind="ExternalInput")
    km16 = nc.dram_tensor("km16", [128, P], mybir.dt.float16,
                          kind="ExternalInput")
    km8x = nc.dram_tensor("km8x", [128, P], mybir.dt.float16,
                          kind="ExternalInput")
    km8 = nc.dram_tensor("km8", [128, 3 * P], mybir.dt.float16,
                         kind="ExternalInput")
    # per pair: [128, D] fp16, partition h*64+p = batch 2p+h, predictor p
    out = nc.dram_tensor("out", [NPAIR * 128, D], mybir.dt.float16,
                         kind="ExternalOutput")

    with tile.TileContext(nc) as tc, ExitStack() as ctx:
        kpool = ctx.enter_context(tc.tile_pool(name="km", bufs=1))
        wpool = ctx.enter_context(tc.tile_pool(name="warm", bufs=1))
        zqpool = ctx.enter_context(tc.tile_pool(name="zq", bufs=NPAIR))
        z8pool = ctx.enter_context(tc.tile_pool(name="z8", bufs=NPAIR))
        opool = ctx.enter_context(tc.tile_pool(name="o", bufs=NPAIR))
        ppool = ctx.enter_context(tc.tile_pool(name="ps", bufs=7, space="PSUM"))
        wppool = ctx.enter_context(tc.tile_pool(name="wps", bufs=1, space="PSUM"))

        # Warm-up: garbage matmuls with no data dependencies so the HAM
        # clock-gate lifts the 1.2 GHz cap before the real matmuls start.
        warm = wpool.tile([128, P + 512], mybir.dt.float16)
        nc.vector.memset(warm[:, :], 1.0)
        wps = wppool.tile([P, 512], mybir.dt.float32)
        for _ in range(NWARM):
            nc.tensor.matmul(wps[:, :], warm[:, :P], warm[:, P : P + 512],
                             start=True, stop=True)

        km16_sb = kpool.tile([128, P], mybir.dt.float16)
        km8x_sb = kpool.tile([128, P], mybir.dt.float16)
        km8_sb = kpool.tile([128, 3 * P], mybir.dt.float16)
        # weights ride gpsimd: keeps their completion receipts off the two
        # HWDGE z-queues (completion sems serialize per queue and lag the
        # data by 1-2.5us under HBM load)
        nc.gpsimd.dma_start(km16_sb[:, :], km16[:, :])
        nc.gpsimd.dma_start(km8x_sb[:, :], km8x[:, :])
        nc.gpsimd.dma_start(km8_sb[:, :], km8[:, :])

        z16t = [zqpool.tile([128, D], mybir.dt.float16,
                            name=f"z16_{i}", tag="z16")
                for i in range(NPAIR // 2)]
        z8xt = [zqpool.tile([128, D], mybir.dt.float8e4,
                            name=f"z8x_{i}", tag="z8x")
                for i in range(NPAIR // 2)]
        z8t = [z8pool.tile([128, 6 * D], mybir.dt.float8e4,
                           name=f"z8_{pr}", tag="z8")
               for pr in range(NPAIR)]

        def _r(pr):
            return slice(pr * 128, (pr + 1) * 128)

        # Input schedule: whole tiles, pair-ordered, both queues balanced at
        # ~1.92MB. Quad tiles (fp16 + fp8x) all land by ~13.5us so the cheap
        # 32-row matmuls can bridge PE gaps; the last items are the big z8
        # tiles of pairs 2/3.
        # Each z8 tile rides ONE queue as a 512KB (q0,q1) piece + 256KB q2
        # piece: staggered completion sems let each pair's q0/q1 matmuls
        # start ~2us before the whole tile's receipt would fire.
        nc.scalar.dma_start(z8xt[0][:, :], z8x[_r(0), :])
        nc.scalar.dma_start(z16t[0][:, :], zj16[_r(0), :])
        nc.sync.dma_start(z8t[0][:, : 4 * D], z8[_r(0), : 4 * D])
        nc.sync.dma_start(z8t[0][:, 4 * D :], z8[_r(0), 4 * D :])
        nc.sync.dma_start(z16t[1][:, :], zj16[_r(1), :])
        nc.sync.dma_start(z8xt[1][:, :], z8x[_r(1), :])
        nc.scalar.dma_start(z8t[1][:, : 4 * D], z8[_r(1), : 4 * D])
        nc.scalar.dma_start(z8t[1][:, 4 * D :], z8[_r(1), 4 * D :])
        nc.sync.dma_start(z8t[2][:, : 4 * D], z8[_r(2), : 4 * D])
        nc.scalar.dma_start(z8t[3][:, : 4 * D], z8[_r(3), : 4 * D])
        nc.sync.dma_start(z8t[2][:, 4 * D :], z8[_r(2), 4 * D :])
        nc.scalar.dma_start(z8t[3][:, 4 * D :], z8[_r(3), 4 * D :])

        def _mm_small(pr, n, which, ps, start, stop=False):
            # 32-row K-tiles: 4 batches of the quad on disjoint 32-row
            # groups; h=0/1 of this pair also on disjoint column groups.
            i, t = pr // 2, pr % 2
            src = z16t[i] if which == 0 else z8xt[i]
            w = km16_sb if which == 0 else km8x_sb
            for h in range(2):
                b = t * 2 + h
                bs = slice(b * 32, (b + 1) * 32)
                nc.tensor.matmul(ps[h * P : (h + 1) * P, :], w[bs, :],
                                 src[bs, n * 512 : (n + 1) * 512],
                                 start=start, stop=stop,
                                 skip_group_check=True,
                                 tile_position=(b * 32, h * P))

        def _mm_big(pr, n, q, ps, start, stop):
            # 128-row fp8 K-tile: h=0/1 in column groups 0-63/64-127 with
            # separate XBUS streams -> concurrent
            for h in range(2):
                rhs = z8t[pr][:, q * 2 * D + h * D + n * 512 :
                              q * 2 * D + h * D + (n + 1) * 512]
                nc.tensor.matmul(ps[h * P : (h + 1) * P, :],
                                 km8_sb[:, q * P : (q + 1) * P], rhs,
                                 start=start, stop=stop,
                                 skip_group_check=True)

        # PE emission in two waves of two pairs. Within a wave, ALL small
        # 32-row matmuls go first (their quad tiles land early, keeping the
        # PE continuously busy so the HAM clock-gate stays at 2.4 GHz), then
        # the big fp8 matmuls chase their tiles as they land. Groups open on
        # the first small matmul and close on q2.
        out_sbs = [opool.tile([128, D], mybir.dt.float16,
                              name=f"o_{pr}", tag="o")
                   for pr in range(NPAIR)]
        pss = {}
        for wave in range(2):
            prs = (0, 1) if wave == 0 else (2, 3)
            for pr in prs:
                for n in range(2):
                    ps = ppool.tile([128, 512], mybir.dt.float32,
                                    name=f"ps{pr}_{n}", tag="ps")
                    pss[pr, n] = ps
                    _mm_small(pr, n, 0, ps, start=True)
                    _mm_small(pr, n, 1, ps, start=False)
            for pr in prs:
                # q0/q1 matmuls of both n-halves first (their 512KB piece's
                # sem fires ~2us before the q2 piece's), q2 closes each group
                for n in range(2):
                    for q in range(2):
                        _mm_big(pr, n, q, pss[pr, n], start=False, stop=False)
                for n in range(2):
                    ps = pss[pr, n]
                    _mm_big(pr, n, 2, ps, start=False, stop=True)
                    if pr == NPAIR - 1 and n == 1:
                        # last cast on scalar ACTIVATE so it overlaps the
                        # DVE's pr3-n0 cast; emitted late so the one-time
                        # ACT table load schedules into scalar's mid-stream
                        # idle, not ahead of its z DMAs
                        nc.scalar.activation(
                            out_sbs[pr][:, 512:], ps[:, :],
                            mybir.ActivationFunctionType.Copy)
                    else:
                        nc.vector.tensor_copy(
                            out_sbs[pr][:, n * 512 : (n + 1) * 512], ps[:, :])
                if pr < NPAIR - 1:
                    nc.gpsimd.dma_start(out[_r(pr), :], out_sbs[pr][:, :])
                else:
                    # last pair: split across the two HWDGE queues
                    nc.sync.dma_start(out[_r(pr), :512], out_sbs[pr][:, :512])
                    nc.scalar.dma_start(out[_r(pr), 512:], out_sbs[pr][:, 512:])

    nc.finalize()
    return nc


def kernel(z_enc: np.ndarray, _trace: bool = False):
    global _NC
    z_enc = np.asarray(z_enc, dtype=np.float32)
    if _NC is None:
        _NC = _build()

    zc = z_enc[:, :C, :]
    # quad tiles: [B, 32, D] -> [B/4 * 128, D] (batch-of-quad outer, k inner)
    zj16 = np.ascontiguousarray(
        zc[:, COLS16, :].astype(np.float16).reshape(B // 4 * 128, D))
    z8x = np.ascontiguousarray(
        zc[:, COLS8X, :].astype(F8).reshape(B // 4 * 128, D))

    # big fp8 tile, q-major: [B,384,D] -> [B/2, 2(h), 3(q), 128(k), D]
    #            -> [B/2, 128(k), 3(q), 2(h), D] -> [B/2*128, 6D]
    z8 = np.ascontiguousarray(
        zc[:, COLS8BIG, :].reshape(B // 2, 2, 3, 128, D)
        .transpose(0, 3, 2, 1, 4)
    ).astype(F8).reshape(B // 2 * 128, 6 * D)

    rows = NPAIR * 128
    qrows = (NPAIR // 2) * 128
    in_maps = [
        {
            "zj16": zj16[i * qrows : (i + 1) * qrows],
            "z8x": z8x[i * qrows : (i + 1) * qrows],
            "z8": z8[i * rows : (i + 1) * rows],
            "km16": KM16Q,
            "km8x": KM8XQ,
            "km8": KM8_DEV,
        }
        for i in range(N_CORES)
    ]

    res = run_bass_kernel_spmd(_NC, in_maps, core_ids=list(range(N_CORES)),
                               trace=_trace)
    # out rows: pair-major, partition h*64+p -> [B/2, 2, 64, D] == [B, 64, D]
    out = np.concatenate(
        [r["out"].reshape(BPC // 2, 128, D) for r in res.results], axis=0
    ).reshape(B, P, D).astype(np.float32)
    if _trace:
        return out, res
    return out


# revision 38
# speedup vs baseline: 1.1924x; 1.0366x over previous
"""GP prediction kernel for Trainium2 (8 NeuronCores, data-parallel over batch).

Computes z_pred[b, p, d] = sum_c k_mult[p, c] * z_enc[b, c, d] where k_mult
is the [64, 448] GP weight matrix k_pred.T @ inv(cov + sigma*I). k_mult
depends only on compile-time constants, so it is precomputed on host; the
device work is a batched [64,448] @ [448,1024] matmul, sharded 8 batches
per core.

Mixed precision against the 2e-2 correctness gate: the PE supports
mixed-dtype matmuls (fp16 stationary x fp8 moving, HW-verified exact), so
ALL weights stay fp16 (zero weight quantization error) and only z is
quantized: the 32 highest-energy context steps travel fp16, the other 416
as unscaled fp8-e4m3. End-to-end error ~1.31e-2, all of it from fp8(z).

PE layout: weights are only 64 wide, so each matmul uses half the
128-wide array; the two batches of a pair run CONCURRENTLY via PE column
tiling (h=0 in array cols 0-63 -> PSUM partitions 0-63, h=1 in cols
64-127) with separate XBUS streams, halving PE streaming time. The two
32-row K-tiles (fp16 + fp8 leftover) pack FOUR batches (two pairs) into
one 128-partition tile on disjoint 32-row groups, so those matmuls also
run on disjoint sub-arrays. PSUM accumulates the 5 K-tiles (32 fp16 +
3x128 fp8 + 32 fp8) into a [128, 512] bank per (pair, 512-col half); one
DVE copy casts each to fp16.

DMA: big pair-ordered transfers on the two HWDGE queues (sync/scalar),
balanced at ~1.92MB each. DMA completion semaphores lag the last byte by
1-2.5us under HBM load (write-receipt round trip), so each 768KB fp8
tile is laid out q-major and sent as a 512KB (q0,q1) piece plus a 256KB
q2 piece on ONE queue: the first piece's sem fires ~2us earlier and the
pair's q0/q1 matmuls overlap the remaining stream. Fine-splitting beyond
this was measured strictly worse. Weights ride the gpsimd (SWDGE) queue
(keeps their receipts off the z-queues), as do outputs for pairs 0-2;
the last pair's output splits across sync+scalar, and its second cast
runs on scalar ACTIVATE to overlap the DVE cast. Warm-up matmuls
(memset on DVE) keep the PE busy from the end of the framework preamble
so the HAM clock-gate lifts to 2.4 GHz before the real matmuls start.
"""
import numpy as np
from contextlib import ExitStack

import concourse.bacc as bacc
import concourse.tile as tile
from concourse import mybir
from concourse.bass_utils import run_bass_kernel_spmd

# Problem constants (hardcoded per harness contract).
B, T, D = 64, 512, 1024
P = 64                 # N_PREDICTORS
C = T - P              # 448 context timesteps
L, SIGMA, TIMESCALE = 0.01, 0.01, 0.3
N_CORES = 8
BPC = B // N_CORES     # batches per core
NPAIR = BPC // 2       # batch pairs per core

N16 = 32               # high-energy columns in fp16 (one 32-row K-tile)
N8 = C - N16           # low-energy columns in fp8: 3x128 + one 32-row tile
NWARM = 9              # garbage matmuls to pull the HAM clock-gate early

F8 = mybir.dt.np(mybir.dt.float8e4)   # ml_dtypes.float8_e4m3


def _k_mult() -> np.ndarray:
    """[P, C] GP weight matrix, solved in float64 on host."""
    t = np.linspace(0.0, 1.0, T)
    t_in = t[:C] * TIMESCALE
    t_pred = t[C:] * TIMESCALE

    def rbf_np(x, y):
        d = x[:, None] - y[None, :]
        return np.exp(-0.5 * d * d / L)

    cov = rbf_np(t_in, t_in) + np.eye(C) * SIGMA
    return np.linalg.solve(cov, rbf_np(t_in, t_pred)).T   # [P, C] float64


def _prep_constants():
    km = _k_mult()
    energy = (km * km).sum(axis=0)
    order = np.argsort(energy)
    cols8 = np.sort(order[:N8])        # fp8 columns, natural order
    cols16 = np.sort(order[N8:])       # fp16 columns (32)
    cols8big = cols8[: 3 * 128]        # the three 128-row fp8 K-tiles
    cols8x = cols8[3 * 128 :]          # leftover 32-row fp8 K-tile

    def quad(w):                       # [32, P] -> [128, P], 4 row-copies
        return np.ascontiguousarray(np.tile(w, (4, 1)).astype(np.float16))

    km16q = quad(km[:, cols16].T)      # [128, P] fp16
    km8xq = quad(km[:, cols8x].T)      # [128, P] fp16
    km8 = np.zeros((128, 3 * P), np.float16)
    for q in range(3):
        km8[:, q * P : (q + 1) * P] = km[:, cols8big[q * 128 : (q + 1) * 128]].T
    return cols16, cols8big, cols8x, km16q, km8xq, km8


COLS16, COLS8BIG, COLS8X, KM16Q, KM8XQ, KM8_DEV = _prep_constants()

_NC = None


def _build():
    nc = bacc.Bacc()
    # per quad (2 pairs): [128, D], partition b*32+k = batch 4i+b, K-row k
    zj16 = nc.dram_tensor("zj16", [(NPAIR // 2) * 128, D], mybir.dt.float16,
                          kind="ExternalInput")
    z8x = nc.dram_tensor("z8x", [(NPAIR // 2) * 128, D], mybir.dt.float8e4,
                         kind="ExternalInput")
    # per pair: [128, 6*D] fp8, col q*2D + h*D + d = subtile q, batch h
    # (q-major so a pair splits into a 512KB q0+q1 piece and a 256KB q2
    # piece whose completion sems stagger ~2us apart)
    z8 = nc.dram_tensor("z8", [NPAIR * 128, 6 * D], mybir.dt.float8e4,
                        kind="ExternalInput")
    km16 = nc.dram_tensor("km16", [128, P], mybir.dt.float16,
                          kind="ExternalInput")
    km8x = nc.dram_tensor("km8x", [128, P], mybir.dt.float16,
                          kind="ExternalInput")
    km8 = nc.dram_tensor("km8", [128, 3 * P], mybir.dt.float16,
                         kind="ExternalInput")
    # per pair: [128, D] fp16, partition h*64+p = batch 2p+h, predictor p
    out = nc.dram_tensor("out", [NPAIR * 128, D], mybir.dt.float16,
                         kind="ExternalOutput")

    with tile.TileContext(nc) as tc, ExitStack() as ctx:
        kpool = ctx.enter_context(tc.tile_pool(name="km", bufs=1))
        wpool = ctx.enter_context(tc.tile_pool(name="warm", bufs=1))
        zqpool = ctx.enter_context(tc.tile_pool(name="zq", bufs=NPAIR))
        z8pool = ctx.enter_context(tc.tile_pool(name="z8", bufs=NPAIR))
        opool = ctx.enter_context(tc.tile_pool(name="o", bufs=NPAIR))
        ppool = ctx.enter_context(tc.tile_pool(name="ps", bufs=7, space="PSUM"))
        wppool = ctx.enter_context(tc.tile_pool(name="wps", bufs=1, space="PSUM"))

        # Warm-up: garbage matmuls with no data dependencies so the HAM
        # clock-gate lifts the 1.2 GHz cap before the real matmuls start.
        warm = wpool.tile([128, P + 512], mybir.dt.float16)
        nc.vector.memset(warm[:, :], 1.0)
        wps = wppool.tile([P, 512], mybir.dt.float32)
        for _ in range(NWARM):
            nc.tensor.matmul(wps[:, :], warm[:, :P], warm[:, P : P + 512],
                             start=True, stop=True)

        km16_sb = kpool.tile([128, P], mybir.dt.float16)
        km8x_sb = kpool.tile([128, P], mybir.dt.float16)
        km8_sb = kpool.tile([128, 3 * P], mybir.dt.float16)
        # weights ride gpsimd: keeps their completion receipts off the two
        # HWDGE z-queues (completion sems serialize per queue and lag the
        # data by 1-2.5us under HBM load)
        nc.gpsimd.dma_start(km16_sb[:, :], km16[:, :])
        nc.gpsimd.dma_start(km8x_sb[:, :], km8x[:, :])
        nc.gpsimd.dma_start(km8_sb[:, :], km8[:, :])

        z16t = [zqpool.tile([128, D], mybir.dt.float16,
                            name=f"z16_{i}", tag="z16")
                for i in range(NPAIR // 2)]
        z8xt = [zqpool.tile([128, D], mybir.dt.float8e4,
                            name=f"z8x_{i}", tag="z8x")
                for i in range(NPAIR // 2)]
        z8t = [z8pool.tile([128, 6 * D], mybir.dt.float8e4,
                           name=f"z8_{pr}", tag="z8")
               for pr in range(NPAIR)]

        def _r(pr):
            return slice(pr * 128, (pr + 1) * 128)

        # Input schedule: whole tiles, pair-ordered, both queues balanced at
        # ~1.92MB. Quad tiles (fp16 + fp8x) all land by ~13.5us so the cheap
        # 32-row matmuls can bridge PE gaps; the last items are the big z8
        # tiles of pairs 2/3.
        # Each z8 tile rides ONE queue as a 512KB (q0,q1) piece + 256KB q2
        # piece: staggered completion sems let each pair's q0/q1 matmuls
        # start ~2us before the whole tile's receipt would fire.
        nc.scalar.dma_start(z8xt[0][:, :], z8x[_r(0), :])
        nc.scalar.dma_start(z16t[0][:, :], zj16[_r(0), :])
        nc.sync.dma_start(z8t[0][:, : 4 * D], z8[_r(0), : 4 * D])
        nc.sync.dma_start(z8t[0][:, 4 * D :], z8[_r(0), 4 * D :])
        nc.sync.dma_start(z16t[1][:, :], zj16[_r(1), :])
        nc.sync.dma_start(z8xt[1][:, :], z8x[_r(1), :])
        nc.scalar.dma_start(z8t[1][:, : 4 * D], z8[_r(1), : 4 * D])
        nc.scalar.dma_start(z8t[1][:, 4 * D :], z8[_r(1), 4 * D :])
        nc.sync.dma_start(z8t[2][:, : 4 * D], z8[_r(2), : 4 * D])
        nc.scalar.dma_start(z8t[3][:, : 4 * D], z8[_r(3), : 4 * D])
        nc.sync.dma_start(z8t[2][:, 4 * D :], z8[_r(2), 4 * D :])
        nc.scalar.dma_start(z8t[3][:, 4 * D :], z8[_r(3), 4 * D :])

        def _mm_small(pr, n, which, ps, start, stop=False):
            # 32-row K-tiles: 4 batches of the quad on disjoint 32-row
            # groups; h=0/1 of this pair also on disjoint column groups.
            i, t = pr // 2, pr % 2
            src = z16t[i] if which == 0 else z8xt[i]
            w = km16_sb if which == 0 else km8x_sb
            for h in range(2):
                b = t * 2 + h
                bs = slice(b * 32, (b + 1) * 32)
                nc.tensor.matmul(ps[h * P : (h + 1) * P, :], w[bs, :],
                                 src[bs, n * 512 : (n + 1) * 512],
                                 start=start, stop=stop,
                                 skip_group_check=True,
                                 tile_position=(b * 32, h * P))

        def _mm_big(pr, n, q, ps, start, stop):
            # 128-row fp8 K-tile: h=0/1 in column groups 0-63/64-127 with
            # separate XBUS streams -> concurrent
            for h in range(2):
                rhs = z8t[pr][:, q * 2 * D + h * D + n * 512 :
                              q * 2 * D + h * D + (n + 1) * 512]
                nc.tensor.matmul(ps[h * P : (h + 1) * P, :],
                                 km8_sb[:, q * P : (q + 1) * P], rhs,
                                 start=start, stop=stop,
                                 skip_group_check=True)

        # PE emission in two waves of two pairs. Within a wave, ALL small
        # 32-row matmuls go first (their quad tiles land early, keeping the
        # PE continuously busy so the HAM clock-gate stays at 2.4 GHz), then
        # the big fp8 matmuls chase their tiles as they land. Groups open on
        # the first small matmul and close on q2.
        out_sbs = [opool.tile([128, D], mybir.dt.float16,
                              name=f"o_{pr}", tag="o")
                   for pr in range(NPAIR)]
        pss = {}
        for wave in range(2):
            prs = (0, 1) if wave == 0 else (2, 3)
            for pr in prs:
                for n in range(2):
                    ps = ppool.tile([128, 512], mybir.dt.float32,
                                    name=f"ps{pr}_{n}", tag="ps")
                    pss[pr, n] = ps
                    _mm_small(pr, n, 0, ps, start=True)
                    _mm_small(pr, n, 1, ps, start=False)
            for pr in prs:
                # q0/q1 matmuls of both n-halves first (their 512KB piece's
                # sem fires ~2us before the q2 piece's), q2 closes each group
                for n in range(2):
                    for q in range(2):
                        _mm_big(pr, n, q, pss[pr, n], start=False, stop=False)
                for n in range(2):
                    ps = pss[pr, n]
                    _mm_big(pr, n, 2, ps, start=False, stop=True)
                    if pr == NPAIR - 1 and n == 1:
                        # last cast on scalar ACTIVATE so it overlaps the
                        # DVE's pr3-n0 cast; emitted late so the one-time
                        # ACT table load schedules into scalar's mid-stream
                        # idle, not ahead of its z DMAs
                        nc.scalar.activation(
                            out_sbs[pr][:, 512:], ps[:, :],
                            mybir.ActivationFunctionType.Copy)
                    else:
                        nc.vector.tensor_copy(
                            out_sbs[pr][:, n * 512 : (n + 1) * 512], ps[:, :])
                if pr < NPAIR - 1:
                    nc.gpsimd.dma_start(out[_r(pr), :], out_sbs[pr][:, :])
                else:
                    # last pair: split across the two HWDGE queues
                    nc.sync.dma_start(out[_r(pr), :512], out_sbs[pr][:, :512])
                    nc.scalar.dma_start(out[_r(pr), 512:], out_sbs[pr][:, 512:])

    nc.finalize()
    return nc


def kernel(z_enc: np.ndarray, _trace: bool = False):
    global _NC
    z_enc = np.asarray(z_enc, dtype=np.float32)
    if _NC is None:
        _NC = _build()

    zc = z_enc[:, :C, :]
    # quad tiles: [B, 32, D] -> [B/4 * 128, D] (batch-of-quad outer, k inner)
    zj16 = np.ascontiguousarray(
        zc[:, COLS16, :].astype(np.float16).reshape(B // 4 * 128, D))
    z8x = np.ascontiguousarray(
        zc[:, COLS8X, :].astype(F8).reshape(B // 4 * 128, D))

    # big fp8 tile, q-major: [B,384,D] -> [B/2, 2(h), 3(q), 128(k), D]
    #            -> [B/2, 128(k), 3(q), 2(h), D] -> [B/2*128, 6D]
    z8 = np.ascontiguousarray(
        zc[:, COLS8BIG, :].reshape(B // 2, 2, 3, 128, D)
        .transpose(0, 3, 2, 1, 4)
    ).astype(F8).reshape(B // 2 * 128, 6 * D)

    rows = NPAIR * 128
    qrows = (NPAIR // 2) * 128
    in_maps = [
        {
            "zj16": zj16[i * qrows : (i + 1) * qrows],
            "z8x": z8x[i * qrows : (i + 1) * qrows],
            "z8": z8[i * rows : (i + 1) * rows],
            "km16": KM16Q,
            "km8x": KM8XQ,
            "km8": KM8_DEV,
        }
        for i in range(N_CORES)
    ]

    res = run_bass_kernel_spmd(_NC, in_maps, core_ids=list(range(N_CORES)),
                               trace=_trace)
    # out rows: pair-major, partition h*64+p -> [B/2, 2, 64, D] == [B, 64, D]
    out = np.concatenate(
        [r["out"].reshape(BPC // 2, 128, D) for r in res.results], axis=0
    ).reshape(B, P, D).astype(np.float32)
    if _trace:
        return out, res
    return out
